# revision 5
# baseline (speedup 1.0000x reference)
"""WENO5 2D advection (Advection3D) Trainium2 kernel.

Full inputs h, u, v: [32, 1024, 1024] f32.  Output: same shape;
out[1:-1, 2:-2, 2:-2] = -div(WENO5 fluxes), 0 on the frame.

Sharding: z-levels across 8 cores (pure data parallel, no halo in z).
Per-core SPMD program processes ZPC=4 z-levels; each z-level is swept in
y-chunks of 128 rows (122 valid output rows per chunk).  Within a chunk:
  - x-direction flux via free-dim shifted access patterns,
  - y-direction flux via DMA SBUF->SBUF partition-shifted copies,
  - divergence combine, DMA out.

Math restructure (validated vs reference in fp32):
  D_j = q_{j+1}-q_j ; A_j = D_j - D_{j-1}
  G0_j = c1312*A_j^2 + .25*(A_j+2D_j)^2      (b0_L(i)=G0_{i-1}, b2_R(i)=G0_i)
  G1_j = c1312*A_j^2 + .25*(D_j+D_{j-1})^2   (b1_L(i)=G1_i, b1_R(i)=G1_{i+1})
  G2_j = c1312*A_j^2 + .25*(A_j-2D_{j-1})^2  (b2_L(i)=G2_{i+1}, b0_R(i)=G2_{i+2})
  B_k = (eps+G_k)^2 ; PP12_j=B1_j*B2_{j+1}; PP01_j=B0_{j-1}*B1_j;
  PP02_j=B0_{j-1}*B2_{j+1}
  denL*10 = PP12+6*PP02+3*PP01 ; denR*10 = PP01+6*PP02+3*PP12 (R read at i+1)
  numL*12 = g0L+2.4(g1L+g2L): g0L=PP12_i*dl0L, g1L=PP02_i*dl1L, g2L=PP01_i*dl2L
  numR*12 = g0R+2.4(g1R+g2R): g0R=PP01_{i+1}*dl0R, g1R=PP02_{i+1}*dl1R,
            g2R=PP12_{i+1}*dl2R
  qL = q_i + (5/6)*numL/denL ; qR = q_{i+1} - (5/6)*numR/denR
  flux = vel*qR + relu(vel)*(qL-qR)
"""
import math

import numpy as np

import concourse.bass as bass
import concourse.mybir as mybir
import concourse.tile as tile

F32 = mybir.dt.float32
ALU = mybir.AluOpType
AF = mybir.ActivationFunctionType

NZ, NY, NX = 32, 1024, 1024
NCORES = 8
ZPC = 4                      # z-levels per core (SPMD-uniform)
PY, PX = NY + 2, NX + 2      # edge-padded
DX = 1000.0
DY = 1000.0
WENO_EPS = 1e-6
C1312S = math.sqrt(13.0 / 12.0)
CHUNK = 122                  # valid output rows per 128-row chunk


class LegalTileContext(tile.TileContext):
    """Tile + wait legalization: this walrus packs at most ONE semaphore wait
    per instruction; hoist extras onto standalone EventSemaphore instructions
    (what raw-bass wait_ge emits)."""

    def _commit_instruction(self, inst, lazy_reg_writes=True):
        si = inst.sync_info
        if si is not None and len(si.on_wait) > 1:
            waits = list(si.on_wait)
            for w in waits[:-1]:
                ev = mybir.InstEventSemaphore(
                    name=f"W-{self.nc.next_id()}", ins=[], outs=[]
                )
                ev.engine = inst.engine
                ev.sync_info = mybir.SyncInfo(on_wait=[w], on_update=[])
                if inst.debug is not None:
                    ev.debug = inst.debug
                super()._commit_instruction(ev, lazy_reg_writes=False)
            inst.sync_info = mybir.SyncInfo(
                on_wait=[waits[-1]], on_update=list(si.on_update)
            )
        return super()._commit_instruction(inst, lazy_reg_writes)

    def _drain_and_barrier(self, tick_clock, wait_clock):
        from concourse.vector_clock import ScopedClock

        nop0 = self.nc.sync.nop()
        wait_clock.add_sem_waits(
            nop0.ins, ScopedClock({None: tick_clock.global_clock})
        )
        si = nop0.ins.sync_info
        if si is not None and len(si.on_wait) > 1:
            waits = list(si.on_wait)
            nop0.ins.sync_info = mybir.SyncInfo(
                on_wait=[waits[0]], on_update=list(si.on_update)
            )
            for w in waits[1:]:
                nopk = self.nc.sync.nop()
                nopk.ins.sync_info = mybir.SyncInfo(on_wait=[w], on_update=[])
        self.nc.sync.drain()

        self.nc.all_engine_barrier()
        assert self.sems is not None
        popped = self.nc._tile_sem_poison_stack.pop()
        assert popped is self._sem_poison
        self.nc.clear_and_free_semaphores(list(self.sems.allocated().values()))
        self.nc.all_engine_barrier()


class Scratch:
    """Rotating scratch-tile allocator over a fixed set of pool tags."""

    def __init__(self, pool, n_tags, shape):
        self.pool = pool
        self.n = n_tags
        self.shape = shape
        self.i = 0

    def __call__(self):
        t = self.pool.tile(self.shape, F32, tag=f"s{self.i % self.n}")
        self.i += 1
        return t


def _emit_direction_x(nc, sc, wk, Q, U):
    """X-direction WENO flux + divergence part. Returns dfex tile
    (valid rows all, cols [3:1023])."""
    tt = nc.vector.tensor_tensor
    stt = nc.vector.scalar_tensor_tensor
    act = nc.scalar.activation

    W = PX  # 1026
    Dx = sc()
    tt(Dx[:, 0 : W - 1], Q[:, 1:W], Q[:, 0 : W - 1], ALU.subtract)
    Ax = sc()
    tt(Ax[:, 1 : W - 1], Dx[:, 1 : W - 1], Dx[:, 0 : W - 2], ALU.subtract)
    t0 = sc()
    stt(t0[:, 1 : W - 1], Dx[:, 1 : W - 1], 2.0, Ax[:, 1 : W - 1], ALU.mult, ALU.add)
    t1 = sc()
    stt(t1[:, 1 : W - 1], Dx[:, 0 : W - 2], -2.0, Ax[:, 1 : W - 1], ALU.mult, ALU.add)
    s = sc()
    tt(s[:, 1 : W - 1], Dx[:, 1 : W - 1], Dx[:, 0 : W - 2], ALU.add)
    lo, hi = 2, W - 3  # face cols [2..1022]
    def V(t, off=0):
        return t[:, lo + off : hi + off]

    dl0L = sc()
    stt(V(dl0L), Dx[:, lo - 2 : hi - 2], -0.4, Dx[:, lo - 1 : hi - 1], ALU.mult, ALU.add)
    dl1L = sc()
    stt(V(dl1L), Dx[:, lo - 1 : hi - 1], 0.5, Dx[:, lo:hi], ALU.mult, ALU.add)
    dl2L = sc()
    stt(V(dl2L), Dx[:, lo + 1 : hi + 1], -0.25, Dx[:, lo:hi], ALU.mult, ALU.add)
    dl0R = sc()
    stt(V(dl0R), Dx[:, lo + 2 : hi + 2], -0.4, Dx[:, lo + 1 : hi + 1], ALU.mult, ALU.add)
    dl1R = sc()
    stt(V(dl1R), Dx[:, lo + 1 : hi + 1], 0.5, Dx[:, lo:hi], ALU.mult, ALU.add)
    dl2R = sc()
    stt(V(dl2R), Dx[:, lo - 1 : hi - 1], -0.25, Dx[:, lo:hi], ALU.mult, ALU.add)
    asq = sc()
    act(asq[:, 1 : W - 1], Ax[:, 1 : W - 1], AF.Square, scale=C1312S)
    q0 = sc()
    act(q0[:, 1 : W - 1], t0[:, 1 : W - 1], AF.Square, scale=0.5)
    q1 = sc()
    act(q1[:, 1 : W - 1], s[:, 1 : W - 1], AF.Square, scale=0.5)
    q2 = sc()
    act(q2[:, 1 : W - 1], t1[:, 1 : W - 1], AF.Square, scale=0.5)
    c0 = sc()
    stt(c0[:, 1 : W - 1], asq[:, 1 : W - 1], WENO_EPS, q0[:, 1 : W - 1], ALU.add, ALU.add)
    c1 = sc()
    stt(c1[:, 1 : W - 1], asq[:, 1 : W - 1], WENO_EPS, q1[:, 1 : W - 1], ALU.add, ALU.add)
    c2 = sc()
    stt(c2[:, 1 : W - 1], asq[:, 1 : W - 1], WENO_EPS, q2[:, 1 : W - 1], ALU.add, ALU.add)
    B0 = sc()
    act(B0[:, 1 : W - 1], c0[:, 1 : W - 1], AF.Square)
    B1 = sc()
    act(B1[:, 1 : W - 1], c1[:, 1 : W - 1], AF.Square)
    B2 = sc()
    act(B2[:, 1 : W - 1], c2[:, 1 : W - 1], AF.Square)
    PP12 = sc()
    tt(PP12[:, 1 : W - 2], B1[:, 1 : W - 2], B2[:, 2 : W - 1], ALU.mult)
    PP01 = sc()
    tt(PP01[:, 2 : W - 1], B0[:, 1 : W - 2], B1[:, 2 : W - 1], ALU.mult)
    PP02 = sc()
    tt(PP02[:, 2 : W - 2], B0[:, 1 : W - 3], B2[:, 3 : W - 1], ALU.mult)
    d1 = sc()
    stt(d1[:, 2 : W - 2], PP02[:, 2 : W - 2], 6.0, PP12[:, 2 : W - 2], ALU.mult, ALU.add)
    denL = sc()
    stt(denL[:, 2 : W - 2], PP01[:, 2 : W - 2], 3.0, d1[:, 2 : W - 2], ALU.mult, ALU.add)
    d2 = sc()
    stt(d2[:, 2 : W - 2], PP02[:, 2 : W - 2], 6.0, PP01[:, 2 : W - 2], ALU.mult, ALU.add)
    denR = sc()
    stt(denR[:, 2 : W - 2], PP12[:, 2 : W - 2], 3.0, d2[:, 2 : W - 2], ALU.mult, ALU.add)

    g0L = sc(); tt(V(g0L), V(PP12), V(dl0L), ALU.mult)
    g1L = sc(); tt(V(g1L), V(PP02), V(dl1L), ALU.mult)
    g2L = sc(); tt(V(g2L), V(PP01), V(dl2L), ALU.mult)
    n1L = sc(); tt(V(n1L), V(g1L), V(g2L), ALU.add)
    numL = sc(); stt(V(numL), V(n1L), 2.4, V(g0L), ALU.mult, ALU.add)
    g0R = sc(); tt(V(g0R), PP01[:, lo + 1 : hi + 1], V(dl0R), ALU.mult)
    g1R = sc(); tt(V(g1R), PP02[:, lo + 1 : hi + 1], V(dl1R), ALU.mult)
    g2R = sc(); tt(V(g2R), PP12[:, lo + 1 : hi + 1], V(dl2R), ALU.mult)
    n1R = sc(); tt(V(n1R), V(g1R), V(g2R), ALU.add)
    numR = sc(); stt(V(numR), V(n1R), 2.4, V(g0R), ALU.mult, ALU.add)

    dp = sc(); tt(V(dp), V(denL), denR[:, lo + 1 : hi + 1], ALU.mult)
    rp = sc(); nc.vector.reciprocal(V(rp), V(dp))
    uL = sc(); tt(V(uL), V(numL), denR[:, lo + 1 : hi + 1], ALU.mult)
    tL = sc(); tt(V(tL), V(uL), V(rp), ALU.mult)
    rL = sc(); stt(V(rL), V(tL), 5.0 / 6.0, Q[:, lo:hi], ALU.mult, ALU.add)
    uR = sc(); tt(V(uR), V(numR), V(denL), ALU.mult)
    tR = sc(); tt(V(tR), V(uR), V(rp), ALU.mult)
    rR = sc(); stt(V(rR), V(tR), -5.0 / 6.0, Q[:, lo + 1 : hi + 1], ALU.mult, ALU.add)

    pU = sc(); act(V(pU), U[:, lo:hi], AF.Relu)
    ds = sc(); tt(V(ds), V(rL), V(rR), ALU.subtract)
    m = sc(); tt(V(m), V(pU), V(ds), ALU.mult)
    fe0 = sc(); tt(V(fe0), U[:, lo:hi], V(rR), ALU.mult)
    fe = sc(); tt(V(fe), V(fe0), V(m), ALU.add)
    # dfex[k] = fe[k] - fe[k-1], out cols [3..1022].  Dedicated tag: dfex
    # stays live across the whole y-phase (rotating-tag reuse would force a
    # scheduling cycle).
    dfex = wk.tile([128, PX], F32, tag="dfex")
    tt(dfex[:, 3 : W - 3], fe[:, 3 : W - 3], fe[:, 2 : W - 4], ALU.subtract)
    return dfex


def _emit_direction_y(nc, sc, wk, Q, V_, qs1):
    """Y-direction WENO flux + divergence part.  Compute ops must start at
    partition 0, so every y-shift is a DMA SBUF->SBUF partition-shifted copy
    (with 1-2 row edge fills so no row is ever uninitialized); all compute
    runs on the full 128 partitions and edge rows carry garbage that the
    final DMA-out range discards."""
    tt = nc.vector.tensor_tensor
    stt = nc.vector.scalar_tensor_tensor
    act = nc.scalar.activation
    dma = nc.sync.dma_start

    W = PX
    A = slice(0, W)

    def shift_down(dst, src, k):
        # dst[p] = src[p-k]; rows [0:k] filled with src[0:k] (garbage-safe)
        dma(dst[k:128, A], src[0 : 128 - k, A])
        dma(dst[0:k, A], src[0:k, A])

    def shift_up(dst, src, k):
        # dst[p] = src[p+k]; rows [128-k:128] filled from src tail
        dma(dst[0 : 128 - k, A], src[k:128, A])
        dma(dst[128 - k : 128, A], src[128 - k : 128, A])

    Dy = sc()
    tt(Dy[:, A], qs1[:, A], Q[:, A], ALU.subtract)          # valid rows [0..126]
    Dm1 = sc(); shift_down(Dm1, Dy, 1)                      # valid [1..127]
    Dm2 = sc(); shift_down(Dm2, Dy, 2)                      # valid [2..127]
    Dp1 = sc(); shift_up(Dp1, Dy, 1)                        # valid [0..125]
    Dp2 = sc(); shift_up(Dp2, Dy, 2)                        # valid [0..124]
    Ay = sc()
    tt(Ay[:, A], Dy[:, A], Dm1[:, A], ALU.subtract)         # valid [1..126]
    t0 = sc()
    stt(t0[:, A], Dy[:, A], 2.0, Ay[:, A], ALU.mult, ALU.add)
    t1 = sc()
    stt(t1[:, A], Dm1[:, A], -2.0, Ay[:, A], ALU.mult, ALU.add)
    s = sc()
    tt(s[:, A], Dy[:, A], Dm1[:, A], ALU.add)

    dl0L = sc(); stt(dl0L[:, A], Dm2[:, A], -0.4, Dm1[:, A], ALU.mult, ALU.add)
    dl1L = sc(); stt(dl1L[:, A], Dm1[:, A], 0.5, Dy[:, A], ALU.mult, ALU.add)
    dl2L = sc(); stt(dl2L[:, A], Dp1[:, A], -0.25, Dy[:, A], ALU.mult, ALU.add)
    dl0R = sc(); stt(dl0R[:, A], Dp2[:, A], -0.4, Dp1[:, A], ALU.mult, ALU.add)
    dl1R = sc(); stt(dl1R[:, A], Dp1[:, A], 0.5, Dy[:, A], ALU.mult, ALU.add)
    dl2R = sc(); stt(dl2R[:, A], Dm1[:, A], -0.25, Dy[:, A], ALU.mult, ALU.add)

    asq = sc(); act(asq[:, A], Ay[:, A], AF.Square, scale=C1312S)
    q0 = sc(); act(q0[:, A], t0[:, A], AF.Square, scale=0.5)
    q1 = sc(); act(q1[:, A], s[:, A], AF.Square, scale=0.5)
    q2 = sc(); act(q2[:, A], t1[:, A], AF.Square, scale=0.5)
    c0 = sc(); stt(c0[:, A], asq[:, A], WENO_EPS, q0[:, A], ALU.add, ALU.add)
    c1 = sc(); stt(c1[:, A], asq[:, A], WENO_EPS, q1[:, A], ALU.add, ALU.add)
    c2 = sc(); stt(c2[:, A], asq[:, A], WENO_EPS, q2[:, A], ALU.add, ALU.add)
    B0 = sc(); act(B0[:, A], c0[:, A], AF.Square)
    B1 = sc(); act(B1[:, A], c1[:, A], AF.Square)
    B2 = sc(); act(B2[:, A], c2[:, A], AF.Square)
    B0m1 = sc(); shift_down(B0m1, B0, 1)                    # valid [2..127]
    B2p1 = sc(); shift_up(B2p1, B2, 1)                      # valid [1..125]
    PP12 = sc(); tt(PP12[:, A], B1[:, A], B2p1[:, A], ALU.mult)   # [1..125]
    PP01 = sc(); tt(PP01[:, A], B0m1[:, A], B1[:, A], ALU.mult)   # [2..126]
    PP02 = sc(); tt(PP02[:, A], B0m1[:, A], B2p1[:, A], ALU.mult) # [2..125]
    d1 = sc()
    stt(d1[:, A], PP02[:, A], 6.0, PP12[:, A], ALU.mult, ALU.add)
    denL = sc()
    stt(denL[:, A], PP01[:, A], 3.0, d1[:, A], ALU.mult, ALU.add) # [2..125]
    d2 = sc()
    stt(d2[:, A], PP02[:, A], 6.0, PP01[:, A], ALU.mult, ALU.add)
    denR = sc()
    stt(denR[:, A], PP12[:, A], 3.0, d2[:, A], ALU.mult, ALU.add) # [2..125]

    PPaR = sc(); shift_up(PPaR, PP01, 1)                    # [1..125]
    PPbR = sc(); shift_up(PPbR, PP02, 1)                    # [1..124]
    PPcR = sc(); shift_up(PPcR, PP12, 1)                    # [0..124]
    denRs = sc(); shift_up(denRs, denR, 1)                  # [1..124]

    g0L = sc(); tt(g0L[:, A], PP12[:, A], dl0L[:, A], ALU.mult)
    g1L = sc(); tt(g1L[:, A], PP02[:, A], dl1L[:, A], ALU.mult)
    g2L = sc(); tt(g2L[:, A], PP01[:, A], dl2L[:, A], ALU.mult)
    n1L = sc(); tt(n1L[:, A], g1L[:, A], g2L[:, A], ALU.add)
    numL = sc(); stt(numL[:, A], n1L[:, A], 2.4, g0L[:, A], ALU.mult, ALU.add)
    g0R = sc(); tt(g0R[:, A], PPaR[:, A], dl0R[:, A], ALU.mult)
    g1R = sc(); tt(g1R[:, A], PPbR[:, A], dl1R[:, A], ALU.mult)
    g2R = sc(); tt(g2R[:, A], PPcR[:, A], dl2R[:, A], ALU.mult)
    n1R = sc(); tt(n1R[:, A], g1R[:, A], g2R[:, A], ALU.add)
    numR = sc(); stt(numR[:, A], n1R[:, A], 2.4, g0R[:, A], ALU.mult, ALU.add)

    dp = sc(); tt(dp[:, A], denL[:, A], denRs[:, A], ALU.mult)    # [2..124]
    rp = sc(); nc.vector.reciprocal(rp[:, A], dp[:, A])
    uL = sc(); tt(uL[:, A], numL[:, A], denRs[:, A], ALU.mult)
    tL = sc(); tt(tL[:, A], uL[:, A], rp[:, A], ALU.mult)
    rL = sc(); stt(rL[:, A], tL[:, A], 5.0 / 6.0, Q[:, A], ALU.mult, ALU.add)
    uR = sc(); tt(uR[:, A], numR[:, A], denL[:, A], ALU.mult)
    tR = sc(); tt(tR[:, A], uR[:, A], rp[:, A], ALU.mult)
    rR = sc(); stt(rR[:, A], tR[:, A], -5.0 / 6.0, qs1[:, A], ALU.mult, ALU.add)

    pV = sc(); act(pV[:, A], V_[:, A], AF.Relu)
    ds = sc(); tt(ds[:, A], rL[:, A], rR[:, A], ALU.subtract)
    m = sc(); tt(m[:, A], pV[:, A], ds[:, A], ALU.mult)
    fn0 = sc(); tt(fn0[:, A], V_[:, A], rR[:, A], ALU.mult)
    fn = sc(); tt(fn[:, A], fn0[:, A], m[:, A], ALU.add)          # [2..124]
    fnm1 = sc(); shift_down(fnm1, fn, 1)                          # [3..125]
    dfny = sc()
    tt(dfny[:, A], fn[:, A], fnm1[:, A], ALU.subtract)            # [3..124]
    return dfny


def build_nc(zpc=ZPC, n_chunks=9):
    nc = bass.Bass()
    h_ext = nc.declare_dram_parameter("h", [zpc, PY, PX], F32, isOutput=False)
    u_ext = nc.declare_dram_parameter("u", [zpc, PY, PX], F32, isOutput=False)
    v_ext = nc.declare_dram_parameter("v", [zpc, PY, PX], F32, isOutput=False)
    o_ext = nc.declare_dram_parameter("o", [zpc, NY, NX], F32, isOutput=True)

    with LegalTileContext(nc) as tc:
        with (
            tc.tile_pool(name="inp", bufs=2) as inp,
            tc.tile_pool(name="wk", bufs=1) as wk,
            tc.tile_pool(name="outp", bufs=2) as outp,
        ):
            sc = Scratch(wk, 32, [128, PX])
            for z in range(zpc):
                for ci in range(n_chunks):
                    r0 = CHUNK * ci
                    if r0 + 128 > PY:
                        r0 = PY - 128
                    Q = inp.tile([128, PX], F32, tag="Q")
                    nc.sync.dma_start(Q[:], h_ext[z, r0 : r0 + 128, :])
                    U = inp.tile([128, PX], F32, tag="U")
                    nc.sync.dma_start(U[:], u_ext[z, r0 : r0 + 128, :])
                    V_ = inp.tile([128, PX], F32, tag="V")
                    nc.sync.dma_start(V_[:], v_ext[z, r0 : r0 + 128, :])
                    qs1 = wk.tile([128, PX], F32, tag="qs1")
                    nc.sync.dma_start(qs1[0:127, :], Q[1:128, :])
                    nc.sync.dma_start(qs1[127:128, :], Q[127:128, :])

                    dfex = _emit_direction_x(nc, sc, wk, Q, U)
                    dfny = _emit_direction_y(nc, sc, wk, Q, V_, qs1)

                    # combine: out = -(dfex/DX + dfny/DY)
                    oc = sc()
                    nc.vector.scalar_tensor_tensor(
                        oc[:, 3 : PX - 3],
                        dfny[:, 3 : PX - 3],
                        DX / DY,
                        dfex[:, 3 : PX - 3],
                        ALU.mult,
                        ALU.add,
                    )
                    oc2 = outp.tile([128, PX], F32, tag="oc2")
                    nc.scalar.activation(
                        oc2[:, 3 : PX - 3],
                        oc[:, 3 : PX - 3],
                        AF.Copy,
                        scale=-1.0 / DX,
                    )
                    # tile row p -> global y = r0 + p - 1; rows p in [3..124]
                    gy0 = r0 + 2
                    nc.sync.dma_start(
                        o_ext[z, gy0 : gy0 + 122, 2 : NX - 2],
                        oc2[3:125, 3 : PX - 3],
                    )
    return nc


_nc_cache = {}


def _get_nc(zpc=ZPC, n_chunks=9):
    key = (zpc, n_chunks)
    if key not in _nc_cache:
        _nc_cache[key] = build_nc(zpc, n_chunks)
    return _nc_cache[key]


def kernel(h, u, v):
    from concourse.bass_utils import run_bass_kernel_spmd

    h = np.asarray(h, dtype=np.float32)
    u = np.asarray(u, dtype=np.float32)
    v = np.asarray(v, dtype=np.float32)
    hp = np.pad(h, ((0, 0), (1, 1), (1, 1)), mode="edge")
    up = np.pad(u, ((0, 0), (1, 1), (1, 1)), mode="edge")
    vp = np.pad(v, ((0, 0), (1, 1), (1, 1)), mode="edge")

    # z-levels 1..30 need computing; pad to 8*4 with repeats of level 30
    levels = list(range(1, NZ - 1)) + [NZ - 2, NZ - 2]
    nc = _get_nc()
    core_ids = list(range(NCORES))
    in_maps = []
    for c in core_ids:
        lv = levels[c * ZPC : (c + 1) * ZPC]
        in_maps.append(
            {
                "h": np.ascontiguousarray(hp[lv]),
                "u": np.ascontiguousarray(up[lv]),
                "v": np.ascontiguousarray(vp[lv]),
            }
        )
    res = run_bass_kernel_spmd(nc, in_maps, core_ids)
    out = np.zeros((NZ, NY, NX), dtype=np.float32)
    for c in core_ids:
        lv = levels[c * ZPC : (c + 1) * ZPC]
        o = res.results[c]["o"]
        for j, z in enumerate(lv):
            out[z, 2 : NY - 2, 2 : NX - 2] = o[j][2 : NY - 2, 2 : NX - 2]
    return out


def profile_once(inputs):
    """Run with trace=True to extract device exec time (ns), if available."""
    from concourse.bass_utils import run_bass_kernel_spmd

    h = np.asarray(inputs["h"], dtype=np.float32)
    u = np.asarray(inputs["u"], dtype=np.float32)
    v = np.asarray(inputs["v"], dtype=np.float32)
    hp = np.pad(h, ((0, 0), (1, 1), (1, 1)), mode="edge")
    up = np.pad(u, ((0, 0), (1, 1), (1, 1)), mode="edge")
    vp = np.pad(v, ((0, 0), (1, 1), (1, 1)), mode="edge")
    levels = list(range(1, NZ - 1)) + [NZ - 2, NZ - 2]
    nc = _get_nc()
    core_ids = list(range(NCORES))
    in_maps = []
    for c in core_ids:
        lv = levels[c * ZPC : (c + 1) * ZPC]
        in_maps.append(
            {
                "h": np.ascontiguousarray(hp[lv]),
                "u": np.ascontiguousarray(up[lv]),
                "v": np.ascontiguousarray(vp[lv]),
            }
        )
    res = run_bass_kernel_spmd(nc, in_maps, core_ids, trace=True)
    return res.exec_time_ns


# revision 17
# speedup vs baseline: 3883.9611x; 3883.9611x over previous
"""WENO5 2D advection (Advection3D) Trainium2 kernel.

Full inputs h, u, v: [32, 1024, 1024] f32.  Output: same shape;
out[1:-1, 2:-2, 2:-2] = -div(WENO5 fluxes), 0 on the frame.

Sharding: z-levels across 8 cores (pure data parallel, no halo in z).
Per-core SPMD program processes ZPC=4 z-levels; each z-level is swept in
y-chunks of 128 rows (122 valid output rows per chunk).  Within a chunk:
  - x-direction flux via free-dim shifted access patterns,
  - y-direction flux via DMA SBUF->SBUF partition-shifted copies,
  - divergence combine, DMA out.

Math restructure (validated vs reference in fp32):
  D_j = q_{j+1}-q_j ; A_j = D_j - D_{j-1}
  G0_j = c1312*A_j^2 + .25*(A_j+2D_j)^2      (b0_L(i)=G0_{i-1}, b2_R(i)=G0_i)
  G1_j = c1312*A_j^2 + .25*(D_j+D_{j-1})^2   (b1_L(i)=G1_i, b1_R(i)=G1_{i+1})
  G2_j = c1312*A_j^2 + .25*(A_j-2D_{j-1})^2  (b2_L(i)=G2_{i+1}, b0_R(i)=G2_{i+2})
  B_k = (eps+G_k)^2 ; PP12_j=B1_j*B2_{j+1}; PP01_j=B0_{j-1}*B1_j;
  PP02_j=B0_{j-1}*B2_{j+1}
  denL*10 = PP12+6*PP02+3*PP01 ; denR*10 = PP01+6*PP02+3*PP12 (R read at i+1)
  numL*12 = g0L+2.4(g1L+g2L): g0L=PP12_i*dl0L, g1L=PP02_i*dl1L, g2L=PP01_i*dl2L
  numR*12 = g0R+2.4(g1R+g2R): g0R=PP01_{i+1}*dl0R, g1R=PP02_{i+1}*dl1R,
            g2R=PP12_{i+1}*dl2R
  qL = q_i + (5/6)*numL/denL ; qR = q_{i+1} - (5/6)*numR/denR
  flux = vel*qR + relu(vel)*(qL-qR)
"""
import math

import numpy as np

import concourse.bass as bass
import concourse.mybir as mybir
import concourse.tile as tile

F32 = mybir.dt.float32
ALU = mybir.AluOpType
AF = mybir.ActivationFunctionType

NZ, NY, NX = 32, 1024, 1024
NCORES = 8
ZPC = 4                      # z-levels per core (SPMD-uniform)
PY, PX = NY + 2, NX + 2      # edge-padded
DX = 1000.0
DY = 1000.0
WENO_EPS = 1e-6
C1312S = math.sqrt(13.0 / 12.0)
CHUNK = 122                  # valid output rows per 128-row chunk


class LegalTileContext(tile.TileContext):
    """Tile + wait legalization: this walrus packs at most ONE semaphore wait
    per instruction; hoist extras onto standalone EventSemaphore instructions
    (what raw-bass wait_ge emits)."""

    def _commit_instruction(self, inst, lazy_reg_writes=True):
        si = inst.sync_info
        if si is not None and len(si.on_wait) > 1:
            waits = list(si.on_wait)
            for w in waits[:-1]:
                ev = mybir.InstEventSemaphore(
                    name=f"W-{self.nc.next_id()}", ins=[], outs=[]
                )
                ev.engine = inst.engine
                ev.sync_info = mybir.SyncInfo(on_wait=[w], on_update=[])
                if inst.debug is not None:
                    ev.debug = inst.debug
                super()._commit_instruction(ev, lazy_reg_writes=False)
            inst.sync_info = mybir.SyncInfo(
                on_wait=[waits[-1]], on_update=list(si.on_update)
            )
        return super()._commit_instruction(inst, lazy_reg_writes)

    def _drain_and_barrier(self, tick_clock, wait_clock):
        from concourse.vector_clock import ScopedClock

        nop0 = self.nc.sync.nop()
        wait_clock.add_sem_waits(
            nop0.ins, ScopedClock({None: tick_clock.global_clock})
        )
        si = nop0.ins.sync_info
        if si is not None and len(si.on_wait) > 1:
            waits = list(si.on_wait)
            nop0.ins.sync_info = mybir.SyncInfo(
                on_wait=[waits[0]], on_update=list(si.on_update)
            )
            for w in waits[1:]:
                nopk = self.nc.sync.nop()
                nopk.ins.sync_info = mybir.SyncInfo(on_wait=[w], on_update=[])
        self.nc.sync.drain()

        self.nc.all_engine_barrier()
        assert self.sems is not None
        popped = self.nc._tile_sem_poison_stack.pop()
        assert popped is self._sem_poison
        self.nc.clear_and_free_semaphores(list(self.sems.allocated().values()))
        self.nc.all_engine_barrier()


class Scratch:
    """Free-list scratch allocator.  Tags are reused only after an explicit
    free(), which callers place after the tile's last consumer is emitted —
    so slot-wait edges always point backward in emission order and can
    never form a scheduling cycle."""

    def __init__(self, pool, shape, prefix="s"):
        self.pool = pool
        self.shape = shape
        self.prefix = prefix
        self.free_tags = []
        self.n = 0
        self.tag_of = {}

    def __call__(self):
        tag = self.free_tags.pop() if self.free_tags else f"{self.prefix}{self._new()}"
        t = self.pool.tile(self.shape, F32, tag=tag)
        self.tag_of[id(t)] = tag
        return t

    def _new(self):
        self.n += 1
        return self.n - 1

    def free(self, *tiles):
        for t in tiles:
            self.free_tags.append(self.tag_of.pop(id(t)))


def _emit_direction_x(nc, sc, wk, Q, U):
    """X-direction WENO flux + divergence part (free-dim shifts).
    Returns dfex tile (valid rows all, cols [3:1023])."""
    tt = nc.vector.tensor_tensor
    stt = nc.vector.scalar_tensor_tensor
    act = nc.scalar.activation

    W = PX  # 1026
    Dx = sc()
    tt(Dx[:, 0 : W - 1], Q[:, 1:W], Q[:, 0 : W - 1], ALU.subtract)
    Ax = sc()
    tt(Ax[:, 1 : W - 1], Dx[:, 1 : W - 1], Dx[:, 0 : W - 2], ALU.subtract)
    t0 = sc()
    stt(t0[:, 1 : W - 1], Dx[:, 1 : W - 1], 2.0, Ax[:, 1 : W - 1], ALU.mult, ALU.add)
    t1 = sc()
    stt(t1[:, 1 : W - 1], Dx[:, 0 : W - 2], -2.0, Ax[:, 1 : W - 1], ALU.mult, ALU.add)
    s = sc()
    tt(s[:, 1 : W - 1], Dx[:, 1 : W - 1], Dx[:, 0 : W - 2], ALU.add)

    lo, hi = 2, W - 3  # face cols [2..1022]
    def V(t, off=0):
        return t[:, lo + off : hi + off]

    dl0L = sc()
    stt(V(dl0L), Dx[:, lo - 2 : hi - 2], -0.4, Dx[:, lo - 1 : hi - 1], ALU.mult, ALU.add)
    dl1L = sc()
    stt(V(dl1L), Dx[:, lo - 1 : hi - 1], 0.5, Dx[:, lo:hi], ALU.mult, ALU.add)
    dl2L = sc()
    stt(V(dl2L), Dx[:, lo + 1 : hi + 1], -0.25, Dx[:, lo:hi], ALU.mult, ALU.add)
    dl0R = sc()
    stt(V(dl0R), Dx[:, lo + 2 : hi + 2], -0.4, Dx[:, lo + 1 : hi + 1], ALU.mult, ALU.add)
    dl1R = sc()
    stt(V(dl1R), Dx[:, lo + 1 : hi + 1], 0.5, Dx[:, lo:hi], ALU.mult, ALU.add)
    dl2R = sc()
    stt(V(dl2R), Dx[:, lo - 1 : hi - 1], -0.25, Dx[:, lo:hi], ALU.mult, ALU.add)
    sc.free(Dx)

    asq = sc()
    act(asq[:, 1 : W - 1], Ax[:, 1 : W - 1], AF.Square, scale=C1312S)
    sc.free(Ax)
    q0 = sc()
    act(q0[:, 1 : W - 1], t0[:, 1 : W - 1], AF.Square, scale=0.5)
    q1 = sc()
    act(q1[:, 1 : W - 1], s[:, 1 : W - 1], AF.Square, scale=0.5)
    q2 = sc()
    act(q2[:, 1 : W - 1], t1[:, 1 : W - 1], AF.Square, scale=0.5)
    sc.free(t0, t1, s)
    c0 = sc()
    stt(c0[:, 1 : W - 1], asq[:, 1 : W - 1], WENO_EPS, q0[:, 1 : W - 1], ALU.add, ALU.add)
    c1 = sc()
    stt(c1[:, 1 : W - 1], asq[:, 1 : W - 1], WENO_EPS, q1[:, 1 : W - 1], ALU.add, ALU.add)
    c2 = sc()
    stt(c2[:, 1 : W - 1], asq[:, 1 : W - 1], WENO_EPS, q2[:, 1 : W - 1], ALU.add, ALU.add)
    sc.free(asq, q0, q1, q2)
    B0 = sc()
    act(B0[:, 1 : W - 1], c0[:, 1 : W - 1], AF.Square)
    B1 = sc()
    act(B1[:, 1 : W - 1], c1[:, 1 : W - 1], AF.Square)
    B2 = sc()
    act(B2[:, 1 : W - 1], c2[:, 1 : W - 1], AF.Square)
    sc.free(c0, c1, c2)
    PP12 = sc()
    tt(PP12[:, 1 : W - 2], B1[:, 1 : W - 2], B2[:, 2 : W - 1], ALU.mult)
    PP01 = sc()
    tt(PP01[:, 2 : W - 1], B0[:, 1 : W - 2], B1[:, 2 : W - 1], ALU.mult)
    PP02 = sc()
    tt(PP02[:, 2 : W - 2], B0[:, 1 : W - 3], B2[:, 3 : W - 1], ALU.mult)
    sc.free(B0, B1, B2)
    d1 = sc()
    stt(d1[:, 2 : W - 2], PP02[:, 2 : W - 2], 6.0, PP12[:, 2 : W - 2], ALU.mult, ALU.add)
    denL = sc()
    stt(denL[:, 2 : W - 2], PP01[:, 2 : W - 2], 3.0, d1[:, 2 : W - 2], ALU.mult, ALU.add)
    d2 = sc()
    stt(d2[:, 2 : W - 2], PP02[:, 2 : W - 2], 6.0, PP01[:, 2 : W - 2], ALU.mult, ALU.add)
    denR = sc()
    stt(denR[:, 2 : W - 2], PP12[:, 2 : W - 2], 3.0, d2[:, 2 : W - 2], ALU.mult, ALU.add)
    sc.free(d1, d2)

    g0L = sc(); tt(V(g0L), V(PP12), V(dl0L), ALU.mult)
    g1L = sc(); tt(V(g1L), V(PP02), V(dl1L), ALU.mult)
    g2L = sc(); tt(V(g2L), V(PP01), V(dl2L), ALU.mult)
    sc.free(dl0L, dl1L, dl2L)
    n1L = sc(); tt(V(n1L), V(g1L), V(g2L), ALU.add)
    numL = sc(); stt(V(numL), V(n1L), 2.4, V(g0L), ALU.mult, ALU.add)
    sc.free(g0L, g1L, g2L, n1L)
    g0R = sc(); tt(V(g0R), PP01[:, lo + 1 : hi + 1], V(dl0R), ALU.mult)
    g1R = sc(); tt(V(g1R), PP02[:, lo + 1 : hi + 1], V(dl1R), ALU.mult)
    g2R = sc(); tt(V(g2R), PP12[:, lo + 1 : hi + 1], V(dl2R), ALU.mult)
    sc.free(dl0R, dl1R, dl2R, PP12, PP01, PP02)
    n1R = sc(); tt(V(n1R), V(g1R), V(g2R), ALU.add)
    numR = sc(); stt(V(numR), V(n1R), 2.4, V(g0R), ALU.mult, ALU.add)
    sc.free(g0R, g1R, g2R, n1R)

    e = slice(2, W - 2)
    lnL = sc(); act(lnL[:, e], denL[:, e], AF.Ln)
    rd0L = sc(); act(rd0L[:, e], lnL[:, e], AF.Exp, scale=-1.0)
    lnR = sc(); act(lnR[:, e], denR[:, e], AF.Ln)
    rd0R = sc(); act(rd0R[:, e], lnR[:, e], AF.Exp, scale=-1.0)
    sc.free(lnL, lnR)
    tnL = sc(); tt(tnL[:, e], denL[:, e], rd0L[:, e], ALU.mult)
    wnL = sc(); nc.vector.tensor_scalar(wnL[:, e], tnL[:, e], 2.0, -1.0, ALU.subtract, ALU.mult)
    rdL = sc(); tt(rdL[:, e], wnL[:, e], rd0L[:, e], ALU.mult)
    sc.free(denL, tnL, wnL, rd0L)
    tnR = sc(); tt(tnR[:, e], denR[:, e], rd0R[:, e], ALU.mult)
    wnR = sc(); nc.vector.tensor_scalar(wnR[:, e], tnR[:, e], 2.0, -1.0, ALU.subtract, ALU.mult)
    rdR = sc(); tt(rdR[:, e], wnR[:, e], rd0R[:, e], ALU.mult)
    sc.free(denR, tnR, wnR, rd0R)
    tL = sc(); tt(V(tL), V(numL), V(rdL), ALU.mult)
    rL = sc(); stt(V(rL), V(tL), 5.0 / 6.0, Q[:, lo:hi], ALU.mult, ALU.add)
    sc.free(numL, rdL, tL)
    tR = sc(); tt(V(tR), V(numR), rdR[:, lo + 1 : hi + 1], ALU.mult)
    rR = sc(); stt(V(rR), V(tR), -5.0 / 6.0, Q[:, lo + 1 : hi + 1], ALU.mult, ALU.add)
    sc.free(numR, rdR, tR)

    pU = sc(); act(V(pU), U[:, lo:hi], AF.Relu)
    ds = sc(); tt(V(ds), V(rL), V(rR), ALU.subtract)
    sc.free(rL)
    m = sc(); tt(V(m), V(pU), V(ds), ALU.mult)
    sc.free(pU, ds)
    fe0 = sc(); tt(V(fe0), U[:, lo:hi], V(rR), ALU.mult)
    sc.free(rR)
    fe = sc(); tt(V(fe), V(fe0), V(m), ALU.add)
    sc.free(fe0, m)
    # U pre-scaled by 1/DX on host; reversed diff = negated contribution:
    # dfex[k] = fe[k-1] - fe[k].  Dedicated tag: dfex stays live across the
    # whole y-phase.
    dfex = wk.tile([128, PX], F32, tag="dfex")
    tt(dfex[:, 3 : W - 3], fe[:, 2 : W - 4], fe[:, 3 : W - 3], ALU.subtract)
    sc.free(fe)
    return dfex


# Band matrices (lhsT layout: S[k, p] = coeff of q_k in out_p).
# Validity windows match the old DMA-shift version; edge rows are garbage
# (partial sums) and are discarded by the final DMA-out row range.
BAND_SPECS = [
    ("shp1", {1: 1.0}),                      # 0: out_p = q_{p+1} (also qs1)
    ("ay", {-1: 1.0, 0: -2.0, 1: 1.0}),      # 1: A_p
    ("t0", {-1: 1.0, 0: -4.0, 1: 3.0}),      # 2: t0_p
    ("t1", {-1: 3.0, 0: -4.0, 1: 1.0}),      # 3: t1_p
    ("s", {-1: -1.0, 1: 1.0}),               # 4: s_p
    ("dl0L", {-2: 0.4, -1: -1.4, 0: 1.0}),   # 5
    ("dl1L", {-1: -0.5, 0: -0.5, 1: 1.0}),   # 6
    ("dl2L", {0: -1.0, 1: 1.25, 2: -0.25}),  # 7
    ("dl0R", {1: -1.0, 2: 1.4, 3: -0.4}),    # 8
    ("dl1R", {0: -1.0, 1: 0.5, 2: 0.5}),     # 9
    ("dl2R", {-1: 0.25, 0: -1.25, 1: 1.0}),  # 10
    ("shm1", {-1: 1.0}),                     # 11: out_p = q_{p-1}
]
NBANDS = len(BAND_SPECS)


def make_bands_host():
    """SBUF-layout band matrices: [128 k-partitions, NBANDS*128 cols]."""
    w = np.zeros((128, NBANDS * 128), dtype=np.float32)
    for b, (_, taps) in enumerate(BAND_SPECS):
        for off, coef in taps.items():
            for p in range(128):
                k = p + off
                if 0 <= k < 128:
                    w[k, b * 128 + p] = coef
    return w


YW = 1024  # y-chain column width (2 PSUM banks / 2 matmul panels)


def _emit_direction_y_pe(nc, sc, wk, psc, bands, Q, V_):
    """Y-direction WENO flux via TensorE banded matmuls; ACT squares/recip
    seed; DVE nonlinear chain.  Returns dfny (valid rows [3..124])."""
    tt = nc.vector.tensor_tensor
    stt = nc.vector.scalar_tensor_tensor
    act = nc.scalar.activation
    A = slice(0, YW)

    def pe(src, b):
        pt = psc()
        for c0 in (0, 512):
            nc.tensor.matmul(
                pt[:, c0 : c0 + 512],
                bands[:, b * 128 : (b + 1) * 128],
                src[:, c0 : c0 + 512],
            )
        return pt

    qs1 = wk.tile([128, PX], F32, tag="qs1")  # dedicated: live until rR
    p = pe(Q, 0)
    act(qs1[:, A], p[:, A], AF.Copy)          # q_{p+1}, valid [0..126]
    psc.free(p)
    p = pe(Q, 1)
    asq = sc(); act(asq[:, A], p[:, A], AF.Square, scale=C1312S)
    psc.free(p)
    p = pe(Q, 2)
    q0 = sc(); act(q0[:, A], p[:, A], AF.Square, scale=0.5)
    psc.free(p)
    p = pe(Q, 3)
    q2 = sc(); act(q2[:, A], p[:, A], AF.Square, scale=0.5)
    psc.free(p)
    p = pe(Q, 4)
    q1 = sc(); act(q1[:, A], p[:, A], AF.Square, scale=0.5)
    psc.free(p)
    dls = []
    for b in (5, 6, 7, 8, 9, 10):
        p = pe(Q, b)
        t = sc(); act(t[:, A], p[:, A], AF.Copy)
        psc.free(p)
        dls.append(t)
    dl0L, dl1L, dl2L, dl0R, dl1R, dl2R = dls

    c0 = sc(); stt(c0[:, A], asq[:, A], WENO_EPS, q0[:, A], ALU.add, ALU.add)
    c1 = sc(); stt(c1[:, A], asq[:, A], WENO_EPS, q1[:, A], ALU.add, ALU.add)
    c2 = sc(); stt(c2[:, A], asq[:, A], WENO_EPS, q2[:, A], ALU.add, ALU.add)
    sc.free(asq, q0, q1, q2)
    B0 = sc(); act(B0[:, A], c0[:, A], AF.Square)
    B1 = sc(); act(B1[:, A], c1[:, A], AF.Square)
    B2 = sc(); act(B2[:, A], c2[:, A], AF.Square)
    sc.free(c0, c1, c2)
    pB0m1 = pe(B0, 11)
    B0m1 = sc(); act(B0m1[:, A], pB0m1[:, A], AF.Copy)
    psc.free(pB0m1)
    pB2p1 = pe(B2, 0)
    PP12 = sc(); tt(PP12[:, A], B1[:, A], pB2p1[:, A], ALU.mult)
    PP01 = sc(); tt(PP01[:, A], B0m1[:, A], B1[:, A], ALU.mult)
    PP02 = sc(); tt(PP02[:, A], B0m1[:, A], pB2p1[:, A], ALU.mult)
    psc.free(pB2p1)
    sc.free(B0, B1, B2, B0m1)
    d1 = sc()
    stt(d1[:, A], PP02[:, A], 6.0, PP12[:, A], ALU.mult, ALU.add)
    denL = sc()
    stt(denL[:, A], PP01[:, A], 3.0, d1[:, A], ALU.mult, ALU.add)
    d2 = sc()
    stt(d2[:, A], PP02[:, A], 6.0, PP01[:, A], ALU.mult, ALU.add)
    denR = sc()
    stt(denR[:, A], PP12[:, A], 3.0, d2[:, A], ALU.mult, ALU.add)
    sc.free(d1, d2)

    lnL = sc(); act(lnL[:, A], denL[:, A], AF.Ln)
    rd0L = sc(); act(rd0L[:, A], lnL[:, A], AF.Exp, scale=-1.0)
    lnR = sc(); act(lnR[:, A], denR[:, A], AF.Ln)
    rd0R = sc(); act(rd0R[:, A], lnR[:, A], AF.Exp, scale=-1.0)
    sc.free(lnL, lnR)
    tnL = sc(); tt(tnL[:, A], denL[:, A], rd0L[:, A], ALU.mult)
    wnL = sc(); nc.vector.tensor_scalar(wnL[:, A], tnL[:, A], 2.0, -1.0, ALU.subtract, ALU.mult)
    rdL = sc(); tt(rdL[:, A], wnL[:, A], rd0L[:, A], ALU.mult)
    sc.free(denL, tnL, wnL, rd0L)
    tnR = sc(); tt(tnR[:, A], denR[:, A], rd0R[:, A], ALU.mult)
    wnR = sc(); nc.vector.tensor_scalar(wnR[:, A], tnR[:, A], 2.0, -1.0, ALU.subtract, ALU.mult)
    rdR = sc(); tt(rdR[:, A], wnR[:, A], rd0R[:, A], ALU.mult)
    sc.free(denR, tnR, wnR, rd0R)

    g0L = sc(); tt(g0L[:, A], PP12[:, A], dl0L[:, A], ALU.mult)
    g1L = sc(); tt(g1L[:, A], PP02[:, A], dl1L[:, A], ALU.mult)
    g2L = sc(); tt(g2L[:, A], PP01[:, A], dl2L[:, A], ALU.mult)
    sc.free(dl0L, dl1L, dl2L)
    n1L = sc(); tt(n1L[:, A], g1L[:, A], g2L[:, A], ALU.add)
    numL = sc(); stt(numL[:, A], n1L[:, A], 2.4, g0L[:, A], ALU.mult, ALU.add)
    sc.free(g0L, g1L, g2L, n1L)
    pPPa = pe(PP01, 0)
    g0R = sc(); tt(g0R[:, A], pPPa[:, A], dl0R[:, A], ALU.mult)
    psc.free(pPPa)
    pPPb = pe(PP02, 0)
    g1R = sc(); tt(g1R[:, A], pPPb[:, A], dl1R[:, A], ALU.mult)
    psc.free(pPPb)
    pPPc = pe(PP12, 0)
    g2R = sc(); tt(g2R[:, A], pPPc[:, A], dl2R[:, A], ALU.mult)
    psc.free(pPPc)
    sc.free(dl0R, dl1R, dl2R, PP12, PP01, PP02)
    pRds = pe(rdR, 0)                        # 1/denR at p+1 (PSUM)
    sc.free(rdR)
    n1R = sc(); tt(n1R[:, A], g1R[:, A], g2R[:, A], ALU.add)
    numR = sc(); stt(numR[:, A], n1R[:, A], 2.4, g0R[:, A], ALU.mult, ALU.add)
    sc.free(g0R, g1R, g2R, n1R)

    tL = sc(); tt(tL[:, A], numL[:, A], rdL[:, A], ALU.mult)
    rL = sc(); stt(rL[:, A], tL[:, A], 5.0 / 6.0, Q[:, A], ALU.mult, ALU.add)
    sc.free(numL, rdL, tL)
    tR = sc(); tt(tR[:, A], numR[:, A], pRds[:, A], ALU.mult)
    psc.free(pRds)
    rR = sc(); stt(rR[:, A], tR[:, A], -5.0 / 6.0, qs1[:, A], ALU.mult, ALU.add)
    sc.free(numR, tR)

    pV = sc(); act(pV[:, A], V_[:, A], AF.Relu)
    ds = sc(); tt(ds[:, A], rL[:, A], rR[:, A], ALU.subtract)
    sc.free(rL)
    m = sc(); tt(m[:, A], pV[:, A], ds[:, A], ALU.mult)
    sc.free(pV, ds)
    fn0 = sc(); tt(fn0[:, A], V_[:, A], rR[:, A], ALU.mult)
    sc.free(rR)
    fn = sc(); tt(fn[:, A], fn0[:, A], m[:, A], ALU.add)
    sc.free(fn0, m)
    pFnm1 = pe(fn, 11)
    # V_ pre-scaled by 1/DY on host; reversed diff = negated contribution.
    dfny = sc()
    tt(dfny[:, A], pFnm1[:, A], fn[:, A], ALU.subtract)
    psc.free(pFnm1)
    sc.free(fn)
    return dfny


def build_nc(zpc=ZPC, n_chunks=9, mode="full", repeat=1):
    nc = bass.Bass()
    h_ext = nc.declare_dram_parameter("h", [zpc, PY, PX], F32, isOutput=False)
    u_ext = nc.declare_dram_parameter("u", [zpc, PY, PX], F32, isOutput=False)
    v_ext = nc.declare_dram_parameter("v", [zpc, PY, PX], F32, isOutput=False)
    b_ext = nc.declare_dram_parameter(
        "bands", [128, NBANDS * 128], F32, isOutput=False
    )
    o_ext = nc.declare_dram_parameter("o", [zpc, NY, NX], F32, isOutput=True)

    with LegalTileContext(nc) as tc:
        with (
            tc.tile_pool(name="inp", bufs=2) as inp,
            tc.tile_pool(name="wk", bufs=2) as wk,
            tc.tile_pool(name="outp", bufs=2) as outp,
            tc.tile_pool(name="bnd", bufs=1) as bnd,
            tc.tile_pool(name="ps", bufs=3, space="PSUM") as psum,
        ):
            bands = bnd.tile([128, NBANDS * 128], F32, tag="bands")
            nc.sync.dma_start(bands[:], b_ext[:])
            sc = Scratch(wk, [128, PX])
            psc = Scratch(psum, [128, YW], prefix="p")
            for _rep in range(repeat):
              for z in range(zpc):
                for ci in range(n_chunks):
                    r0 = CHUNK * ci
                    if r0 + 128 > PY:
                        r0 = PY - 128
                    Q = inp.tile([128, PX], F32, tag="Q")
                    nc.sync.dma_start(Q[:], h_ext[z, r0 : r0 + 128, :])
                    U = inp.tile([128, PX], F32, tag="U")
                    nc.sync.dma_start(U[:], u_ext[z, r0 : r0 + 128, :])
                    V_ = inp.tile([128, PX], F32, tag="V")
                    nc.sync.dma_start(V_[:], v_ext[z, r0 : r0 + 128, :])
                    if mode in ("full", "xonly"):
                        dfex = _emit_direction_x(nc, sc, wk, Q, U)
                    if mode in ("full", "yonly"):
                        dfny = _emit_direction_y_pe(
                            nc, sc, wk, psc, bands, Q, V_
                        )

                    oc2 = outp.tile([128, PX], F32, tag="oc2")
                    if mode == "full":
                        # out = dfex' + dfny' (both already negated+scaled)
                        nc.vector.tensor_tensor(
                            oc2[:, 3 : PX - 3],
                            dfny[:, 3 : PX - 3],
                            dfex[:, 3 : PX - 3],
                            ALU.add,
                        )
                        sc.free(dfny)
                    else:
                        src = dfex if mode == "xonly" else (
                            dfny if mode == "yonly" else Q
                        )
                        nc.scalar.activation(
                            oc2[:, 3 : PX - 3], src[:, 3 : PX - 3], AF.Copy
                        )
                        if mode == "yonly":
                            sc.free(dfny)
                    # tile row p -> global y = r0 + p - 1; rows p in [3..124]
                    gy0 = r0 + 2
                    nc.sync.dma_start(
                        o_ext[z, gy0 : gy0 + 122, 2 : NX - 2],
                        oc2[3:125, 3 : PX - 3],
                    )
    import sys
    print(
        f"build_nc: scratch_tags={sc.n} psum_tags={psc.n}",
        file=sys.stderr,
    )
    return nc


_nc_cache = {}


def _get_nc(zpc=ZPC, n_chunks=9, mode="full", repeat=1):
    key = (zpc, n_chunks, mode, repeat)
    if key not in _nc_cache:
        _nc_cache[key] = build_nc(zpc, n_chunks, mode, repeat)
    return _nc_cache[key]


def kernel(h, u, v):
    from concourse.bass_utils import run_bass_kernel_spmd

    h = np.asarray(h, dtype=np.float32)
    u = np.asarray(u, dtype=np.float32)
    v = np.asarray(v, dtype=np.float32)
    hp = np.pad(h, ((0, 0), (1, 1), (1, 1)), mode="edge")
    up = np.pad(u, ((0, 0), (1, 1), (1, 1)), mode="edge") * np.float32(1.0 / DX)
    vp = np.pad(v, ((0, 0), (1, 1), (1, 1)), mode="edge") * np.float32(1.0 / DY)

    # z-levels 1..30 need computing; pad to 8*4 with repeats of level 30
    levels = list(range(1, NZ - 1)) + [NZ - 2, NZ - 2]
    nc = _get_nc()
    core_ids = list(range(NCORES))
    in_maps = []
    for c in core_ids:
        lv = levels[c * ZPC : (c + 1) * ZPC]
        in_maps.append(
            {
                "h": np.ascontiguousarray(hp[lv]),
                "u": np.ascontiguousarray(up[lv]),
                "v": np.ascontiguousarray(vp[lv]),
                "bands": make_bands_host(),
            }
        )
    res = run_bass_kernel_spmd(nc, in_maps, core_ids)
    out = np.zeros((NZ, NY, NX), dtype=np.float32)
    for c in core_ids:
        lv = levels[c * ZPC : (c + 1) * ZPC]
        o = res.results[c]["o"]
        for j, z in enumerate(lv):
            out[z, 2 : NY - 2, 2 : NX - 2] = o[j][2 : NY - 2, 2 : NX - 2]
    return out


def profile_once(inputs):
    """Run with trace=True to extract device exec time (ns), if available."""
    from concourse.bass_utils import run_bass_kernel_spmd

    h = np.asarray(inputs["h"], dtype=np.float32)
    u = np.asarray(inputs["u"], dtype=np.float32)
    v = np.asarray(inputs["v"], dtype=np.float32)
    hp = np.pad(h, ((0, 0), (1, 1), (1, 1)), mode="edge")
    up = np.pad(u, ((0, 0), (1, 1), (1, 1)), mode="edge") * np.float32(1.0 / DX)
    vp = np.pad(v, ((0, 0), (1, 1), (1, 1)), mode="edge") * np.float32(1.0 / DY)
    levels = list(range(1, NZ - 1)) + [NZ - 2, NZ - 2]
    nc = _get_nc()
    core_ids = list(range(NCORES))
    in_maps = []
    for c in core_ids:
        lv = levels[c * ZPC : (c + 1) * ZPC]
        in_maps.append(
            {
                "h": np.ascontiguousarray(hp[lv]),
                "u": np.ascontiguousarray(up[lv]),
                "v": np.ascontiguousarray(vp[lv]),
                "bands": make_bands_host(),
            }
        )
    res = run_bass_kernel_spmd(nc, in_maps, core_ids, trace=True)
    return res.exec_time_ns
